# revision 16
# baseline (speedup 1.0000x reference)
"""Bidirectional-LSTM basecaller on 8 Trainium2 NeuronCores (Bass/Tile).

Sharding: cores 0-3 run the FORWARD LSTM for 8 sequences each; cores 4-7 run
the BACKWARD direction for the same 8-sequence groups, fed host-reversed
signals (conv kernels flipped host-side, which commutes with SAME conv).
Variable sequence lengths are handled uniformly: each backward lane's signal
is rotated so real data starts at step 1 (step 0 consumes one garbage conv
window whose state pollution is killed by adding -40 to the i-gate input
projection at step 0 -> c0 ~ 0, h0 ~ 0). The same program runs on all 8
cores; only input data differs.

Per core phases:
  A conv1 (rank-1 matmul + relu)      -> conv1T fp16 in SBUF
  B conv2 (K=3 as 6 accum. matmuls)   -> conv2T fp16
  C conv3 + conv1a + add              -> encT fp16
  D gx = encT @ Wx + b (bulk matmul)  -> DRAM fp32 (streamed back in chunks)
  E 2049 sequential LSTM steps: DVE pre-writes gx[t] into PSUM, PE
    accumulates 16 [128|72 x 128] x [.,8] recurrent matmuls (fp16, FWL),
    ACT sigmoid/tanh, DVE cell update; h appended to SBUF-resident h-seq
  F dense decode h @ Wd_half -> part [5, TS, 8] -> DRAM

Host: pack shards, run SPMD, assemble logits (bw time-remap + bias + masking).
"""
import os
import sys
import numpy as np

for _p in ("/opt/trn_rl_repo", "/root/.axon_site/_ro/trn_rl_repo"):
    if os.path.isdir(_p) and _p not in sys.path:
        sys.path.insert(0, _p)

import ml_dtypes  # noqa: E402

B, T, H, C = 32, 2048, 200, 256
GH = 256                # padded per-gate width
G4 = 4 * GH             # 1024
LANES = 8
NB = 512                # matmul free-dim block
F16 = ml_dtypes.bfloat16  # overridden below; actual np dtype for 16-bit tensors
NP16 = np.float16       # use IEEE fp16 (10-bit mantissa) for weights/activations

_CACHE = {}


# ---------------------------------------------------------------- bass build
def _build(TS, CT):
    """Build the SPMD bass program for TS LSTM steps, gx chunk size CT."""
    import concourse.bass as bass
    import concourse.tile as tile
    from concourse import bacc, mybir

    f32 = mybir.dt.float32
    f16 = mybir.dt.float16

    SIGP = TS + 2                     # padded signal cols
    NBL = (SIGP + NB - 1) // NB       # signal blocks per lane
    NROWS = LANES * NBL
    KT = [128, H - 128]               # K-tile sizes for hidden contraction (128, 72)

    nc = bacc.Bacc("TRN2", target_bir_lowering=False, debug=False, num_devices=8)

    c1t_d = nc.dram_tensor("c1t", [2, 128, LANES * SIGP], f16, kind="ExternalInput")
    enc0_d = nc.dram_tensor("enc0", [2, 128, LANES * TS], f16, kind="ExternalInput")
    k2_d = nc.dram_tensor("k2", [2, 3, 2, 128, 128], f16, kind="ExternalInput")
    k3_d = nc.dram_tensor("k3", [2, 2, 128, 128], f16, kind="ExternalInput")
    wx_d = nc.dram_tensor("wx", [2, 128, G4], f16, kind="ExternalInput")
    wh_d = nc.dram_tensor("wh", [2, 128, G4], f16, kind="ExternalInput")
    gb_d = nc.dram_tensor("gb", [128, 8], f32, kind="ExternalInput")
    wd_d = nc.dram_tensor("wd", [2, 128, 5], f16, kind="ExternalInput")
    patch_d = nc.dram_tensor("patch", [128, 2, 8], f32, kind="ExternalInput")

    gx_d = nc.dram_tensor("gx", [8, 128, LANES, TS], f32, kind="Internal")
    part_d = nc.dram_tensor("part", [5, TS, LANES], f32, kind="ExternalOutput")

    # t-blocks covering TS columns
    tblks = [(i * NB, min(NB, TS - i * NB)) for i in range((TS + NB - 1) // NB)]
    # gx chunks
    chks = [(i * CT, min(CT, TS - i * CT)) for i in range((TS + CT - 1) // CT)]

    with tile.TileContext(nc) as tc:
        # ---------------- persistent pools
        with (
            tc.tile_pool(name="seq", bufs=1) as seqp,          # c, wh, wd, consts
            tc.tile_pool(name="wts", bufs=1) as wtsp,          # conv/gx weights
            tc.tile_pool(name="stage", bufs=4) as stagep,      # psum->sbuf staging
            tc.tile_pool(name="psA", bufs=4, space="PSUM") as psA,
        ):
            # ---------------- load inputs to SBUF
            k2_s = wtsp.tile([128, 2, 3, 2, 128], f16)
            nc.sync.dma_start(k2_s[:], k2_d.ap().rearrange("a k c p m -> p a k c m"))
            k3_s = wtsp.tile([128, 2, 2, 128], f16)
            nc.sync.dma_start(k3_s[:], k3_d.ap().rearrange("a c p m -> p a c m"))
            wx_s = wtsp.tile([128, 2, G4], f16)
            nc.sync.dma_start(wx_s[:], wx_d.ap().rearrange("k p m -> p k m"))
            gb_s = wtsp.tile([128, 8], f32)
            nc.sync.dma_start(gb_s[:], gb_d.ap())
            wh_s = seqp.tile([128, 2, G4], f16)
            nc.sync.dma_start(wh_s[:], wh_d.ap().rearrange("k p m -> p k m"))
            wd_s = seqp.tile([128, 2, 5], f16)
            nc.sync.dma_start(wd_s[:], wd_d.ap().rearrange("k p m -> p k m"))
            patch_s = seqp.tile([128, 2, 8], f32)
            nc.sync.dma_start(patch_s[:], patch_d.ap())

            with tc.tile_pool(name="conv", bufs=1) as convp:
                # ------------ A: conv1T (host-computed rank-1 conv) DMA in
                conv1T = convp.tile([128, 2, LANES, SIGP], f16, tag="bigA")
                for ci in range(2):
                    nc.sync.dma_start(
                        conv1T[:, ci, :, :].rearrange("p l t -> p (l t)"),
                        c1t_d.ap()[ci])

                # ------------ B: conv2T [128, 2co, 8, TS] f16
                conv2T = convp.tile([128, 2, LANES, TS], f16, tag="bigB")
                for co in range(2):
                    for ln in range(LANES):
                        for o, n in tblks:
                            ps = psA.tile([128, NB], f32, tag="psA")
                            first = True
                            for k in range(3):
                                for ci in range(2):
                                    nc.tensor.matmul(
                                        ps[:, :n], k2_s[:, co, k, ci, :],
                                        conv1T[:, ci, ln, o + k:o + k + n],
                                        start=first, stop=(k == 2 and ci == 1))
                                    first = False
                            nc.scalar.activation(conv2T[:, co, ln, o:o + n],
                                                 ps[:, :n],
                                                 mybir.ActivationFunctionType.Relu)

                # ------------ C: encT = conv1a (preloaded) + relu(conv3)
                encT = convp.tile([128, 2, LANES, TS], f16, tag="bigA")
                for co in range(2):
                    nc.sync.dma_start(
                        encT[:, co, :, :].rearrange("p l t -> p (l t)"),
                        enc0_d.ap()[co])
                for co in range(2):
                    for ln in range(LANES):
                        for o, n in tblks:
                            ps3 = psA.tile([128, NB], f32, tag="psA")
                            for ci in range(2):
                                nc.tensor.matmul(ps3[:, :n], k3_s[:, co, ci, :],
                                                 conv2T[:, ci, ln, o:o + n],
                                                 start=(ci == 0), stop=(ci == 1))
                            t3 = stagep.tile([128, NB], f16, tag="st3")
                            nc.scalar.activation(t3[:, :n], ps3[:, :n],
                                                 mybir.ActivationFunctionType.Relu)
                            nc.vector.tensor_add(encT[:, co, ln, o:o + n],
                                                 encT[:, co, ln, o:o + n],
                                                 t3[:, :n])

                # ------------ D: gx = encT @ Wx + gb -> DRAM [8m,128,8,TS] f32
                for m in range(8):
                    for ln in range(LANES):
                        for o, n in tblks:
                            ps = psA.tile([128, NB], f32, tag="psA")
                            for ci in range(2):
                                nc.tensor.matmul(ps[:, :n],
                                                 wx_s[:, ci, m * 128:(m + 1) * 128],
                                                 encT[:, ci, ln, o:o + n],
                                                 start=(ci == 0), stop=(ci == 1))
                            st = stagep.tile([128, NB], f32, tag="stgx")
                            nc.scalar.activation(
                                st[:, :n], ps[:, :n],
                                mybir.ActivationFunctionType.Identity,
                                bias=gb_s[:, m:m + 1])
                            nc.sync.dma_start(gx_d.ap()[m, :, ln, o:o + n],
                                              st[:, :n])

            # ---------------- E: recurrence (+ F) in their own SBUF scope
            with (
                tc.tile_pool(name="big", bufs=1) as bigp,
                tc.tile_pool(name="gxb", bufs=2) as gxbp,
                tc.tile_pool(name="zp", bufs=2, space="PSUM") as zp,
                tc.tile_pool(name="gat", bufs=3) as gatp,
                tc.tile_pool(name="psF", bufs=2, space="PSUM") as psF,
            ):
                hseq = bigp.tile([128, TS + 1, 2, LANES], f16, tag="hseq")
                c_sb = seqp.tile([128, 2, LANES], f32, tag="c")
                nc.vector.memset(hseq[:, 0, :, :], 0.0)
                nc.vector.memset(c_sb[:], 0.0)

                gxbufs = []
                for ci_, (o, n) in enumerate(chks):
                    gxb = gxbp.tile([128, 8, LANES, CT], f32, tag="gxb")
                    for mi in range(8):
                        nc.sync.dma_start(gxb[:, mi, :, :n],
                                          gx_d.ap()[mi, :, :, o:o + n])
                    if ci_ == 0:
                        nc.vector.tensor_add(gxb[:, 2:4, :, 0],
                                             gxb[:, 2:4, :, 0], patch_s[:])
                    gxbufs.append((gxb, o, n))

                for ch, (gxb, o, n) in enumerate(gxbufs):
                    for lt in range(n):
                        u = o + lt
                        z = zp.tile([128, 64], f32, tag="z")
                        nc.vector.tensor_copy(
                            z[:].rearrange("p (m l) -> p m l", m=8),
                            gxb[:, :, :, lt])
                        for m in range(8):
                            for k in range(2):
                                kn = KT[k]
                                nc.tensor.matmul(
                                    z[:, m * 8:(m + 1) * 8],
                                    wh_s[0:kn, k, m * 128:(m + 1) * 128],
                                    hseq[0:kn, u, k, :],
                                    start=False, stop=(k == 1),
                                    skip_group_check=True)
                        gt = gatp.tile([128, 64], f32, tag="gt")
                        nc.scalar.activation(gt[:, 0:16], z[:, 0:16],
                                             mybir.ActivationFunctionType.Tanh)
                        nc.scalar.activation(gt[:, 16:64], z[:, 16:64],
                                             mybir.ActivationFunctionType.Sigmoid)
                        tmp = gatp.tile([128, 16], f32, tag="tmp")
                        nc.vector.tensor_mul(tmp[:], gt[:, 0:16], gt[:, 16:32])
                        nc.vector.tensor_mul(
                            c_sb[:].rearrange("p a l -> p (a l)"),
                            c_sb[:].rearrange("p a l -> p (a l)"),
                            gt[:, 32:48])
                        nc.vector.tensor_add(
                            c_sb[:].rearrange("p a l -> p (a l)"),
                            c_sb[:].rearrange("p a l -> p (a l)"),
                            tmp[:])
                        tc_t = gatp.tile([128, 16], f32, tag="tanc")
                        nc.scalar.activation(
                            tc_t[:], c_sb[:].rearrange("p a l -> p (a l)"),
                            mybir.ActivationFunctionType.Tanh)
                        nc.vector.tensor_mul(
                            hseq[:, u + 1, :, :].rearrange("p a l -> p (a l)"),
                            tc_t[:], gt[:, 48:64])

                # ------------ F: part[:, u, :] = hseq[u+1] @ wd
                DB = 64
                fblks = [(i * DB, min(DB, TS - i * DB))
                         for i in range((TS + DB - 1) // DB)]
                for o, n in fblks:
                    psf = psF.tile([5, NB], f32, tag="psf")
                    for k in range(2):
                        kn = KT[k]
                        nc.tensor.matmul(
                            psf[:, :n * 8].rearrange("p (t l) -> p t l", l=8),
                            wd_s[0:kn, k, :],
                            hseq[0:kn, 1 + o:1 + o + n, k, :],
                            start=(k == 0), stop=(k == 1))
                    stf = stagep.tile([5, NB], f32, tag="stf")
                    nc.vector.tensor_copy(stf[:, :n * 8], psf[:, :n * 8])
                    nc.sync.dma_start(
                        part_d.ap()[:, o:o + n, :],
                        stf[:, :n * 8].rearrange("p (t l) -> p t l", l=8))

    nc.compile()
    return nc


# ---------------------------------------------------------------- host side
def _pack_core(signals, sig_length, k1w, k1aw, k1ab, k2w, k3w,
               Wf, bf, Wb, bb, Wd, bd, core, TS):
    is_bw = core >= 4
    seqs0 = 8 * (core % 4)
    sig = signals[seqs0:seqs0 + 8, :, 0]
    L = sig_length[seqs0:seqs0 + 8].astype(np.int64)

    y = np.zeros((LANES, TS), np.float32)
    if not is_bw:
        y[:, :T] = sig
    else:
        for l in range(LANES):
            Ll = int(L[l])
            y[l, 0] = sig[l, Ll] if Ll < T else 0.0
            y[l, 1:1 + Ll] = sig[l, :Ll][::-1]
    SIGP = TS + 2
    ypad = np.zeros((LANES, SIGP), np.float32)
    ypad[:, 1:TS + 1] = y
    k1 = k1w[0, 0].astype(np.float32)      # [256]
    k1a = k1aw[0, 0].astype(np.float32)
    k1abv = k1ab.astype(np.float32)
    # conv1T [2, 128, LANES*SIGP], enc0 (=conv1a) [2, 128, LANES*TS], fp16
    c1t = np.maximum(k1[:, None, None] * ypad[None], 0.0)
    c1t = c1t.reshape(2, 128, LANES * SIGP).astype(NP16)
    enc0 = np.maximum(k1a[:, None, None] * ypad[None, :, 1:TS + 1]
                      + k1abv[:, None, None], 0.0)
    enc0 = enc0.reshape(2, 128, LANES * TS).astype(NP16)

    W = (Wb if is_bw else Wf).astype(np.float32)
    bvec = (bb if is_bw else bf).astype(np.float32).copy()
    Wx = W[:C]
    Wh = W[C:]
    gsel = [1, 0, 2, 3]  # j, i, f, o
    Wx_p = np.zeros((C, G4), np.float32)
    Wh_p = np.zeros((H, G4), np.float32)
    gb = np.zeros((G4,), np.float32)
    for gi, g in enumerate(gsel):
        Wx_p[:, gi * GH:gi * GH + H] = Wx[:, g * H:(g + 1) * H]
        Wh_p[:, gi * GH:gi * GH + H] = Wh[:, g * H:(g + 1) * H]
        gb[gi * GH:gi * GH + H] = bvec[g * H:(g + 1) * H]
    gb[2 * GH:2 * GH + H] += 1.0

    k2 = (k2w[::-1] if is_bw else k2w).astype(np.float32)   # [3, 256, 256]
    k3 = k3w[0].astype(np.float32)                          # [256, 256]
    Wd_half = (Wd[H:] if is_bw else Wd[:H]).astype(np.float32)  # [200, 5]

    wh_packed = np.zeros((2, 128, G4), NP16)
    wh_packed[0] = Wh_p[0:128]
    wh_packed[1, 0:H - 128] = Wh_p[128:H]
    wd_packed = np.zeros((2, 128, 5), NP16)
    wd_packed[0] = Wd_half[0:128]
    wd_packed[1, 0:H - 128] = Wd_half[128:H]

    patch = np.zeros((128, 2, 8), np.float32)
    if is_bw:
        patch[:] = -40.0

    return {
        "c1t": c1t,
        "enc0": enc0,
        "k2": np.ascontiguousarray(
            k2.reshape(3, 2, 128, 2, 128).transpose(3, 0, 1, 2, 4)).astype(NP16),
        "k3": np.ascontiguousarray(
            k3.reshape(2, 128, 2, 128).transpose(2, 0, 1, 3)).astype(NP16),
        "wx": np.ascontiguousarray(Wx_p.reshape(2, 128, G4)).astype(NP16),
        "wh": wh_packed,
        "gb": np.ascontiguousarray(gb.reshape(8, 128).T).astype(np.float32),
        "wd": wd_packed,
        "patch": patch,
    }, L


def kernel(signals, sig_length, k1w, k1aw, k1ab, k2w, k3w, Wf, bf, Wb, bb, Wd, bd):
    from concourse.bass_utils import run_bass_kernel_spmd

    TS = T + 1
    CT = 128
    signals = np.asarray(signals, np.float32)
    sig_length = np.asarray(sig_length).astype(np.int64)
    args = [np.asarray(a, np.float32) for a in
            (k1w, k1aw, k1ab, k2w, k3w, Wf, bf, Wb, bb, Wd, bd)]
    k1w, k1aw, k1ab, k2w, k3w, Wf, bf, Wb, bb, Wd, bd = args

    key = (TS, CT)
    if key not in _CACHE:
        _CACHE[key] = _build(TS, CT)
    nc = _CACHE[key]

    in_maps = []
    Ls = []
    for core in range(8):
        m, L = _pack_core(signals, sig_length, k1w, k1aw, k1ab, k2w, k3w,
                          Wf, bf, Wb, bb, Wd, bd, core, TS)
        in_maps.append(m)
        Ls.append(L)

    res = run_bass_kernel_spmd(nc, in_maps, core_ids=list(range(8)))
    parts = [res.results[c]["part"] for c in range(8)]

    logits = np.zeros((B, T, 5), np.float32)
    t_idx = np.arange(T)
    for core in range(8):
        part = parts[core]
        seqs0 = 8 * (core % 4)
        for l in range(LANES):
            b = seqs0 + l
            if core < 4:
                logits[b] += part[:, :T, l].T
            else:
                L = int(Ls[core][l])
                u = L - t_idx
                valid = u >= 1
                logits[b, valid] += part[:, u[valid], l].T
    logits += bd[None, None, :].astype(np.float32)
    for b in range(B):
        L = int(sig_length[b])
        logits[b, L:] = bd
    return logits.astype(np.float32)
